# revision 15
# baseline (speedup 1.0000x reference)
"""Trainium2 Bass kernel for CausalQKMemoryProjection.

Math (per batch b, position t, flattened dim D = H*Dh = 1024):
  out_t = (M_p @ q_t + sum_j W[t,j]*g[j]*(q_t . k_j) k_j)
          / (NORM_P + sum_j W[t,j]*g[j]*||k_j||^2 + EPS)
  W[t,j] = gamma^(t-j-1) for 1 <= t-j <= CTX (sliding causal window), else 0.

Sharding: 8 cores = (batch b in 0..3) x (sequence half in 0..1); each core
handles 1024 query positions with a 128-position key halo on the left.

Device layouts per core:
  qT (D, 1024)  : q^T, d-major (MM1 lhsT and qpers lhsT)
  kT (D, 1152)  : k^T halo, d-major (MM1 rhs; transposed on-chip -> kN)
  out (1024, D) : natural (t, d)

Fast path (m_persistent == s*I, the shipped case) is fully DMA-paced:
q/k are loaded in 256-column pieces and the per-block compute is emitted
interleaved with the load stream, so the tensor engine tracks the DMA
front.  Tile's dependency tracking is AP-range precise, so each matmul
waits only for the columns it actually reads.

Per 128-query block qb (software-pipelined):
  MM1   : dots(t,j) [128x256] = sum_d qT^T kT     (8 accum matmuls, fp32r)
  DVE   : copy dots PSUM -> SBUF
  PE    : transpose dots -> dots^T(j,t); norm matmuls (maskT^T @ gknsq)
  DVE   : A^T = dots^T * mgT; recip = 1/(norm + NORM_P + EPS)
  MM2   : out(t,d) += sum_j A^T^T kN  (+ qpers: bf16 diagonal matmuls vs
          s*I in the fast path, or the full fp32r mp matmul otherwise)
  DVE/ACT: out_sb = out_psum * recip -> 2 store DMAs
"""

import numpy as np

B, H, S, Dh = 4, 16, 2048, 64
D = H * Dh            # 1024
CTX = 128
GAMMA = 0.95
NORM_P = D * 0.01     # 10.24
EPS = 1e-6

N_CORES = 8
SLOC = S // 2         # 1024 positions per core
HALO = SLOC + CTX     # 1152 key positions per core
P = 128
NQB = SLOC // P       # 8 query blocks
NDC = D // P          # 8 dim chunks
NJB = HALO // P       # 9 key blocks
PC = 256              # load piece width (columns)


def _make_maskT():
    """maskT (128, 256): [r*128+tt] columns, jj rows; r=0 previous block,
    r=1 diagonal block; t - j = tt - jj + 128*(1-r)."""
    m = np.zeros((P, 2 * P), np.float32)
    jj = np.arange(P)[:, None]
    tt = np.arange(P)[None, :]
    for r in (0, 1):
        delta = tt - jj + 128 * (1 - r)
        w = np.where((delta >= 1) & (delta <= CTX),
                     GAMMA ** np.clip(delta - 1, 0, None).astype(np.float32),
                     0.0)
        m[:, r * P:(r + 1) * P] = w
    return m.astype(np.float32)


_COMPILED = {}


def _build(loop_n=1, mp_mode="fast", out_internal=False):
    import concourse.mybir as mybir
    import concourse.tile as tile
    from concourse import bacc

    F32 = mybir.dt.float32
    F32R = mybir.dt.float32r
    BF16 = mybir.dt.bfloat16

    nc = bacc.Bacc("TRN2", target_bir_lowering=False, debug=False,
                   enable_asserts=False, num_devices=N_CORES)
    qT_d = nc.dram_tensor("qT", (D, SLOC), F32R, kind="ExternalInput").ap()
    kT_d = nc.dram_tensor("kT", (D, HALO), F32R, kind="ExternalInput").ap()
    g_d = nc.dram_tensor("gates", (P, NJB), F32, kind="ExternalInput").ap()
    mask_d = nc.dram_tensor("maskT", (P, 2 * P), F32, kind="ExternalInput").ap()
    eye_d = nc.dram_tensor("eye", (P, P), F32, kind="ExternalInput").ap()
    if mp_mode == "full":
        mp_d = nc.dram_tensor("mp", (D, D), F32R, kind="ExternalInput").ap()
        si_d = None
    else:
        mp_d = None
        si_d = nc.dram_tensor("sI", (P, P), BF16, kind="ExternalInput").ap()
    if out_internal:
        out_d = nc.dram_tensor("out_i", (SLOC, D), F32, kind="Internal").ap()
        dummy_d = nc.dram_tensor("tiny_out", (1, 1), F32,
                                 kind="ExternalOutput").ap()
    else:
        out_d = nc.dram_tensor("out", (SLOC, D), F32,
                               kind="ExternalOutput").ap()
        dummy_d = None

    def body(tc, pools):
        perst, work, small, ps_dots, ps_sh, ps_out = pools

        # tiny constants up front
        mask_sb = perst.tile([P, 2 * P], F32, tag="mask")
        nc.sync.dma_start(out=mask_sb[:], in_=mask_d[:, :])
        eye_sb = perst.tile([P, P], F32, tag="eye")
        nc.sync.dma_start(out=eye_sb[:], in_=eye_d[:, :])
        gates_sb = perst.tile([P, NJB], F32, tag="gates")
        nc.sync.dma_start(out=gates_sb[:], in_=g_d[:, :])
        if si_d is not None:
            si_sb = perst.tile([P, P], BF16, tag="sI")
            nc.sync.dma_start(out=si_sb[:], in_=si_d[:, :])

        # mgT[qb, r] = maskT_r * gate_col(qb + r)   (cheap, DVE, early)
        mg_sb = {}
        for qb in range(NQB):
            for r in (0, 1):
                t = perst.tile([P, P], F32, tag=f"mg{qb}_{r}")
                nc.vector.tensor_scalar_mul(
                    t[:], mask_sb[:, r * P:(r + 1) * P],
                    gates_sb[:, qb + r:qb + r + 1])
                mg_sb[(qb, r)] = t

        # persistent tiles: q/k chunk-stacked along the free dim so one
        # 3D-AP DMA can fill a column piece of every chunk at once
        qT_all = perst.tile([P, NDC * SLOC], F32R, tag="qT_all")
        kT_all = perst.tile([P, NDC * HALO], F32R, tag="kT_all")
        qT_sb = [qT_all[:, ci * SLOC:(ci + 1) * SLOC] for ci in range(NDC)]
        kT_sb = [kT_all[:, ci * HALO:(ci + 1) * HALO] for ci in range(NDC)]
        kN_sb = [perst.tile([P, D], F32R, tag=f"kN{jb}", name=f"kN{jb}")
                 for jb in range(NJB)]
        gknsq_sb = perst.tile([P, NJB], F32, tag="gknsq")
        qT_src = qT_d.rearrange("(c p) s -> p c s", p=P)
        kT_src = kT_d.rearrange("(c p) s -> p c s", p=P)
        qT_dst = qT_all.rearrange("p (c s) -> p c s", c=NDC)
        kT_dst = kT_all.rearrange("p (c s) -> p c s", c=NDC)
        if mp_mode == "fast":
            qTb_all = perst.tile([P, NDC * SLOC], BF16, tag="qTb_all")
            qTb_sb = [qTb_all[:, ci * SLOC:(ci + 1) * SLOC]
                      for ci in range(NDC)]
        else:
            mp_all = perst.tile([P, NDC * D], F32R, tag="mp_all")
            mp_sb = [mp_all[:, ci * D:(ci + 1) * D] for ci in range(NDC)]
        # dots: two 256-wide halves manually packed into one PSUM bank
        dots_all = ps_dots.tile([P, 512], F32, tag="dots_all")

        cp = [0]

        def emit_kn(jb):
            """transpose kT[:, jb-block] -> kN[jb], then gknsq[jb]."""
            for half in (0, 1):
                stage = ps_sh.tile([P, 512], F32, tag="at")
                for u in range(4):
                    ci = half * 4 + u
                    nc.tensor.transpose(
                        stage[:, u * P:(u + 1) * P],
                        kT_sb[ci][:, jb * P:(jb + 1) * P].bitcast(F32),
                        eye_sb[:])
                dst = kN_sb[jb][:, half * 512:(half + 1) * 512]
                if cp[0] % 2 == 0:
                    nc.vector.tensor_copy(dst, stage[:])
                else:
                    nc.scalar.copy(dst, stage[:])
                cp[0] += 1
            sq = work.tile([P, D], F32, tag="sq_scratch")
            col = small.tile([P, 1], F32, tag="knsq_col")
            if jb % 2 == 0:
                nc.scalar.activation(sq[:], kN_sb[jb][:].bitcast(F32),
                                     mybir.ActivationFunctionType.Square,
                                     accum_out=col[:])
            else:
                nc.vector.scalar_tensor_tensor(
                    sq[:], kN_sb[jb][:].bitcast(F32), 1.0,
                    kN_sb[jb][:].bitcast(F32),
                    op0=mybir.AluOpType.mult, op1=mybir.AluOpType.mult,
                    accum_out=col[:])
            nc.vector.tensor_mul(gknsq_sb[:, jb:jb + 1], col[:],
                                 gates_sb[:, jb:jb + 1])

        def emit_mm1(qb):
            dsl = dots_all[:, (qb % 2) * 256:(qb % 2) * 256 + 256]
            for ci in range(NDC):
                nc.tensor.matmul(
                    dsl,
                    qT_sb[ci][:, qb * P:(qb + 1) * P],
                    kT_sb[ci][:, qb * P:qb * P + 2 * P],
                    start=(ci == 0), stop=(ci == NDC - 1))
            return dsl

        def emit_rest(qb, dots_ps):
            dots_sb = work.tile([P, 2 * P], F32, tag="dots_sb")
            nc.vector.tensor_copy(dots_sb[:], dots_ps)

            # at tile: [0:256] transposed dots, [256:257] norm column
            at_ps = ps_sh.tile([P, 264], F32, tag="at")
            for r in (0, 1):
                nc.tensor.transpose(at_ps[:, r * P:(r + 1) * P],
                                    dots_sb[:, r * P:(r + 1) * P], eye_sb[:])
            for r in (0, 1):
                nc.tensor.matmul(at_ps[:, 256:257],
                                 mask_sb[:, r * P:(r + 1) * P],
                                 gknsq_sb[:, qb + r:qb + r + 1],
                                 start=(r == 0), stop=(r == 1))
            ag_sb = []
            for r in (0, 1):
                t = work.tile([P, P], F32R, tag=f"ag{r}", name=f"ag{r}")
                nc.vector.tensor_mul(t[:], at_ps[:, r * P:(r + 1) * P],
                                     mg_sb[(qb, r)][:])
                ag_sb.append(t)
            rec = small.tile([P, 1], F32, tag="rec")
            nc.vector.tensor_scalar_add(rec[:], at_ps[:, 256:257],
                                        NORM_P + EPS)
            rec2 = small.tile([P, 1], F32, tag="rec2")
            nc.vector.reciprocal(rec2[:], rec[:])

            out_ps = ps_out.tile([P, D], F32, tag="out")
            for h in (0, 1):
                sl = slice(h * 512, (h + 1) * 512)
                for r in (0, 1):
                    nc.tensor.matmul(out_ps[:, sl], ag_sb[r][:],
                                     kN_sb[qb + r][:, sl],
                                     start=(r == 0), stop=False)
                if mp_mode == "full":
                    for ci in range(NDC):
                        nc.tensor.matmul(
                            out_ps[:, sl],
                            qT_sb[ci][:, qb * P:(qb + 1) * P],
                            mp_sb[ci][:, sl],
                            start=False, stop=(ci == NDC - 1))
                else:
                    for u in range(4):
                        ci = h * 4 + u
                        csl = slice(ci * P, (ci + 1) * P)
                        nc.tensor.matmul(
                            out_ps[:, csl],
                            qTb_sb[ci][:, qb * P:(qb + 1) * P],
                            si_sb[:],
                            start=False, stop=(u == 3))

            outN = work.tile([P, D], F32, tag="outN")
            nc.vector.tensor_scalar_mul(outN[:, 0:512], out_ps[:, 0:512],
                                        rec2[:])
            nc.scalar.mul(outN[:, 512:1024], out_ps[:, 512:1024], rec2[:])
            nc.sync.dma_start(out=out_d[qb * P:(qb + 1) * P, :], in_=outN[:])

        if mp_mode == "full":
            # mp first so per-block qpers matmuls never stall on it
            nc.sync.dma_start(
                out=mp_all.rearrange("p (c s) -> p c s", c=NDC)[:, :, :],
                in_=mp_d.rearrange("(c p) s -> p c s", p=P)[:, :, :])
        # DMA-paced: 256-column pieces; interleave kN stages and blocks
        NKP = HALO // PC + (1 if HALO % PC else 0)   # 5 kT pieces
        NQP = SLOC // PC                             # 4 qT pieces
        prev = None
        for p in range(NKP):
            klo, khi = p * PC, min((p + 1) * PC, HALO)
            nc.sync.dma_start(out=kT_dst[:, :, klo:khi],
                              in_=kT_src[:, :, klo:khi])
            if p < NQP:
                qlo, qhi = p * PC, (p + 1) * PC
                nc.sync.dma_start(out=qT_dst[:, :, qlo:qhi],
                                  in_=qT_src[:, :, qlo:qhi])
                if mp_mode == "fast":
                    for ci in range(NDC):
                        nc.vector.tensor_copy(
                            qTb_sb[ci][:, qlo:qhi],
                            qT_sb[ci][:, qlo:qhi].bitcast(F32))
            for jb in (2 * p, 2 * p + 1):
                if jb < NJB:
                    emit_kn(jb)
            # blocks that become ready with this piece
            ready = [qb for qb in range(NQB)
                     if max(qb // 2, (qb + 1) // 2) == p]
            for qb in ready:
                dsl = emit_mm1(qb)
                if prev is not None:
                    emit_rest(*prev)
                prev = (qb, dsl)
        emit_rest(*prev)

        if dummy_d is not None:
            nc.sync.dma_start(out=dummy_d[:, :], in_=gates_sb[0:1, 0:1])

    with tile.TileContext(nc) as tc:
        with (
            tc.tile_pool(name="perst", bufs=1) as perst,
            tc.tile_pool(name="work", bufs=2) as work,
            tc.tile_pool(name="small", bufs=2) as small,
            tc.tile_pool(name="ps_dots", bufs=1, space="PSUM") as ps_dots,
            tc.tile_pool(name="ps_sh", bufs=3, space="PSUM") as ps_sh,
            tc.tile_pool(name="ps_out", bufs=2, space="PSUM") as ps_out,
        ):
            pools = (perst, work, small, ps_dots, ps_sh, ps_out)
            if loop_n == 1:
                body(tc, pools)
            else:
                import concourse.mybir as _mb
                hints = (_mb.EngineType.PE, _mb.EngineType.DVE,
                         _mb.EngineType.Activation, _mb.EngineType.SP)
                with tc.For_i(0, loop_n, 1, hint_engines=hints):
                    body(tc, pools)
    nc.compile()
    return nc


def _get_compiled(loop_n=1, mp_mode="fast", out_internal=False):
    key = (loop_n, mp_mode, out_internal)
    if key not in _COMPILED:
        _COMPILED[key] = _build(loop_n, mp_mode, out_internal)
    return _COMPILED[key]


def _mp_scaled_identity(mp):
    """Return scale s if m_persistent == s * I (exactly), else None."""
    mp = np.asarray(mp)
    if mp.shape != (D, D):
        return None
    s = float(mp[0, 0])
    dg = np.diagonal(mp)
    if not np.all(dg == s):
        return None
    if np.count_nonzero(mp) != np.count_nonzero(dg):
        return None
    return s


def _shard_inputs(q, k, gamma_gates, m_persistent):
    """Build the 8 per-core input maps (host-side layout marshaling only)."""
    import ml_dtypes

    q = np.asarray(q, np.float32)
    k = np.asarray(k, np.float32)
    g = np.asarray(gamma_gates, np.float32)
    mp = np.ascontiguousarray(np.asarray(m_persistent, np.float32))
    maskT = _make_maskT()
    eye = np.eye(P, dtype=np.float32)
    s = _mp_scaled_identity(mp)

    in_maps = []
    for c in range(N_CORES):
        b, half = divmod(c, 2)
        t0 = half * SLOC
        qb_ = q[b][:, t0:t0 + SLOC, :]                    # (H, SLOC, Dh)
        qT = np.ascontiguousarray(qb_.transpose(0, 2, 1).reshape(D, SLOC))
        lo = t0 - CTX
        if lo < 0:
            kh = np.concatenate(
                [np.zeros((H, CTX, Dh), np.float32), k[b][:, :t0 + SLOC, :]],
                axis=1)
            gh = np.concatenate(
                [np.zeros((CTX, 1), np.float32), g[b][:t0 + SLOC, :]], axis=0)
        else:
            kh = k[b][:, lo:t0 + SLOC, :]
            gh = g[b][lo:t0 + SLOC, :]
        kT = np.ascontiguousarray(kh.transpose(0, 2, 1).reshape(D, HALO))
        m = {
            "qT": qT, "kT": kT,
            "gates": np.ascontiguousarray(
                gh.reshape(NJB, P).T, np.float32),
            "maskT": maskT, "eye": eye,
        }
        if s is None:
            m["mp"] = mp
        else:
            m["sI"] = (s * eye).astype(ml_dtypes.bfloat16)
        in_maps.append(m)
    return in_maps, ("full" if s is None else "fast")


def kernel(q, k, gamma_gates, m_persistent):
    from concourse.bass_utils import run_bass_kernel_spmd

    in_maps, mp_mode = _shard_inputs(q, k, gamma_gates, m_persistent)
    nc = _get_compiled(1, mp_mode)
    res = run_bass_kernel_spmd(nc, in_maps, core_ids=list(range(N_CORES)))

    out = np.empty((B, H, S, Dh), np.float32)
    for c in range(N_CORES):
        b, half = divmod(c, 2)
        t0 = half * SLOC
        oc = res.results[c]["out"]                         # (SLOC, D)
        out[b, :, t0:t0 + SLOC, :] = oc.reshape(SLOC, H, Dh).transpose(1, 0, 2)
    return out
